# revision 1
# baseline (speedup 1.0000x reference)
"""Multi-head attention (B=2, S=2048, D=1024, H=16) on 8 Trainium2 cores.

Sharding: core c handles batch c//4 and head-group c%4 (4 heads x dk 64).
Q/K/V/O projection weights are column-split by head group on the host.

Attention keeps scores in [k, q] orientation.  The PV product streams the
exp-scores as the wide moving operand against a stationary V slice (plus a
ones column), producing the per-head output directly transposed as
[dk+1, q] with the softmax denominator in row dk.  Normalization:
denominator row -> gpsimd partition-broadcast -> DVE fast reciprocal ->
fused multiply.  Scores for block i+1 are interleaved with the PV matmuls
of block i so PE and ACT stay busy together.

Per-head outputs are exchanged with one AllGather per 512-token chunk
inside each 4-core batch group; each core then runs the output projection
for its own token chunk (selected with a partition-id dynamic slice).
"""

import numpy as np
import ml_dtypes

import concourse.bass as bass
import concourse.tile as tile
from concourse import bacc, mybir
from concourse.bass_utils import run_bass_kernel_spmd

BF16 = mybir.dt.bfloat16
F32 = mybir.dt.float32
NPBF16 = ml_dtypes.bfloat16

B, S, D, H = 2, 2048, 1024, 16
DK = 64
DK1 = DK + 1
N_CORES = 8
HPC = 4               # heads per core
FEAT = HPC * DK       # 256 projected features per core
VW = HPC * DK1        # 260: v with a ones column per head
TOKC = 1024           # token chunk for projections
QCH = 1024            # q block for attention
NKT = S // 128        # 16 k tiles
NKC = D // 128        # 8 contraction chunks
CHUNK = S // 4        # 512-token output chunk per core
NQB = S // QCH        # 2 q blocks

_CACHE = {}


def _build_program(reps=1, skip_ag=False):
    key = ("nc", reps, skip_ag)
    if key in _CACHE:
        return _CACHE[key]

    nc = bacc.Bacc("TRN2", target_bir_lowering=False, debug=False,
                   num_devices=N_CORES)

    xq = nc.declare_dram_parameter("xq", [D, S], BF16, isOutput=False)
    xk = nc.declare_dram_parameter("xk", [D, S], BF16, isOutput=False)
    xv = nc.declare_dram_parameter("xv", [D, S], BF16, isOutput=False)
    wq = nc.declare_dram_parameter("wq", [D, FEAT], BF16, isOutput=False)
    wk = nc.declare_dram_parameter("wk", [D, FEAT], BF16, isOutput=False)
    wv = nc.declare_dram_parameter("wv", [D, VW], BF16, isOutput=False)
    wo = nc.declare_dram_parameter("wo", [D, D], BF16, isOutput=False)
    bq = nc.declare_dram_parameter("bq", [128, 2], F32, isOutput=False)
    bk = nc.declare_dram_parameter("bk", [128, 2], F32, isOutput=False)
    bv = nc.declare_dram_parameter("bv", [1, VW], BF16, isOutput=False)
    bo = nc.declare_dram_parameter("bo", [1, D], BF16, isOutput=False)
    out = nc.declare_dram_parameter("out", [CHUNK, D], F32, isOutput=True)

    MUL = mybir.AluOpType.mult

    with tile.TileContext(nc) as tc:
        with (
            tc.tile_pool(name="w", bufs=1) as wpool,
            tc.tile_pool(name="x", bufs=26) as xpool,
            tc.tile_pool(name="qk", bufs=1) as qkpool,
            tc.tile_pool(name="vp", bufs=1) as vpool,
            tc.tile_pool(name="sc", bufs=18) as scpool,
            tc.tile_pool(name="nm", bufs=2) as nmpool,
            tc.tile_pool(name="sm", bufs=8) as smpool,
            tc.tile_pool(name="cat", bufs=1) as catpool,
            tc.tile_pool(name="fo", bufs=3) as fopool,
            tc.tile_pool(name="ps_sc", bufs=3, space="PSUM") as ps_sc,
            tc.tile_pool(name="ps_pv", bufs=1, space="PSUM") as ps_pv,
            tc.tile_pool(name="dram", bufs=1, space="DRAM") as dram,
        ):
            for _ in range(reps):
                _emit_rep(nc, wpool, xpool, qkpool, vpool, scpool, smpool,
                          nmpool, catpool, fopool, ps_sc, ps_pv, dram,
                          xq, xk, xv, wq, wk, wv, wo, bq, bk, bv, bo, out,
                          MUL, skip_ag)

    nc.compile()
    _CACHE[key] = nc
    return nc


def _emit_rep(nc, wpool, xpool, qkpool, vpool, scpool, smpool, nmpool,
              catpool, fopool, ps_sc, ps_pv, dram,
              xq, xk, xv, wq, wk, wv, wo, bq, bk, bv, bo, out, MUL,
              skip_ag=False):
    NPAIR = S // (2 * CHUNK)  # 2 chunk-pairs
    EXPF = mybir.ActivationFunctionType.Exp

    ones1 = wpool.tile([1, 128], BF16, tag="ones")
    nc.vector.memset(ones1[:], 1.0)

    # DMA emission order is chosen so the first K-projection inputs land
    # first; wo (2 MB, phase-3 only) is requested last, during attention.
    wk_sb, bk_sb = [], None
    for kc in range(NKC):
        t = wpool.tile([128, FEAT], BF16, tag=f"wk{kc}")
        nc.sync.dma_start(t[:], wk[bass.ts(kc, 128), :])
        wk_sb.append(t)
    bk_sb = wpool.tile([128, 2], F32, tag="bk")
    nc.sync.dma_start(bk_sb[:], bk[:])

    qh_sb = [qkpool.tile([128, S], BF16, tag=f"qh{m}", name=f"qh{m}")
             for m in range(2)]
    kh_sb = [qkpool.tile([128, S], BF16, tag=f"kh{m}", name=f"kh{m}")
             for m in range(2)]
    v_sb = [vpool.tile([128, VW], BF16, tag=f"v{j}", name=f"v{j}")
            for j in range(NKT)]

    def load_x(src, t0):
        tiles = []
        for kc in range(NKC):
            t = xpool.tile([128, TOKC], BF16, tag="xt")
            nc.sync.dma_start(t[:], src[bass.ts(kc, 128), bass.ts(t0, TOKC)])
            tiles.append(t)
        return tiles

    def qk_group(w_sb, x_t, b_sb, dst, t0, m):
        ps = ps_sc.tile([128, TOKC], F32, tag="sc")
        for kc in range(NKC):
            for u in range(TOKC // 512):
                nc.tensor.matmul(
                    ps[:, bass.ts(u, 512)],
                    w_sb[kc][:, bass.ts(m, 128)],
                    x_t[kc][:, bass.ts(u, 512)],
                    start=(kc == 0), stop=(kc == NKC - 1),
                )
        nc.vector.tensor_scalar_add(dst[m][:, bass.ts(t0, TOKC)], ps[:],
                                    b_sb[:, m:m + 1])

    # ---- K projection (scores need the full kh) ------------------
    xk_ts = [load_x(xk, t0) for t0 in range(S // TOKC)]
    for t0 in range(S // TOKC):
        for m in range(2):
            qk_group(wk_sb, xk_ts[t0], bk_sb, kh_sb, t0, m)

    wq_sb = []
    for kc in range(NKC):
        t = wpool.tile([128, FEAT], BF16, tag=f"wq{kc}")
        nc.sync.dma_start(t[:], wq[bass.ts(kc, 128), :])
        wq_sb.append(t)
    bq_sb = wpool.tile([128, 2], F32, tag="bq")
    nc.sync.dma_start(bq_sb[:], bq[:])
    for t0 in range(S // TOKC):
        xq_t = load_x(xq, t0)
        for m in range(2):
            qk_group(wq_sb, xq_t, bq_sb, qh_sb, t0, m)

    # ---- V weights + inputs; V projection runs as block-0 fillers
    wv_sb = []
    for kc in range(NKC):
        t = wpool.tile([128, VW], BF16, tag=f"wv{kc}")
        nc.sync.dma_start(t[:], wv[bass.ts(kc, 128), :])
        wv_sb.append(t)
    bv_sb = wpool.tile([1, VW], BF16, tag="bv")
    nc.sync.dma_start(bv_sb[:], bv[:])
    xv_ts = [load_x(xv, t0) for t0 in range(S // TOKC)]

    def v_group(t0, j):
        ps = ps_sc.tile([128, VW], F32, tag="sc")
        for kc in range(NKC):
            nc.tensor.matmul(
                ps[:], xv_ts[t0][kc][:, bass.ts(j, 128)], wv_sb[kc][:],
                start=(kc == 0), stop=False,
            )
        nc.tensor.matmul(ps[:], ones1[:], bv_sb[:], start=False, stop=True)
        nc.vector.tensor_copy(v_sb[t0 * (TOKC // 128) + j][:], ps[:])

    for t0 in range(S // TOKC):
        for j in range(TOKC // 128):
            v_group(t0, j)

    # wo + bo requested now: the 2 MB load drains during attention.
    wo_sb = []
    for kc in range(NKC):
        t = wpool.tile([128, D], BF16, tag=f"wo{kc}")
        nc.sync.dma_start(t[:], wo[bass.ts(kc, 128), :])
        wo_sb.append(t)
    bo_bf = wpool.tile([1, D], BF16, tag="bo")
    nc.sync.dma_start(bo_bf[:], bo[:])

    # ---- phase 2: attention --------------------------------------
    ag_in = [dram.tile([FEAT, CHUNK], BF16, tag=f"agi{c}", name=f"agi{c}")
             for c in range(S // CHUNK)]
    ag_out = dram.tile([4 * D, CHUNK], BF16, tag="ago")

    def norm_and_out(pv, h, qb):
        pvs = nmpool.tile([DK1, QCH], F32, tag="pvs")
        nc.vector.tensor_copy(pvs[:], pv[:])
        drow = nmpool.tile([1, QCH], F32, tag="drow")
        nc.vector.tensor_copy(drow[:], pvs[DK:DK1, :])
        db = nmpool.tile([DK, QCH], F32, tag="db")
        nc.gpsimd.partition_broadcast(db[:], drow[:])
        rb = nmpool.tile([DK, QCH], F32, tag="rb")
        nc.vector.reciprocal_approx_fast(rb[:], db[:])
        onrm = nmpool.tile([DK, QCH], BF16, tag="onrm")
        nc.vector.scalar_tensor_tensor(onrm[:], pvs[0:DK, :], 1.0, rb[:],
                                       MUL, MUL)
        for u in range(QCH // CHUNK):
            cidx = qb * (QCH // CHUNK) + u
            nc.sync.dma_start(
                ag_in[cidx][h * DK:(h + 1) * DK, :],
                onrm[:, bass.ts(u, CHUNK)])
        if h == HPC - 1 and not skip_ag:
            for u in range(QCH // CHUNK):
                cidx = qb * (QCH // CHUNK) + u
                nc.gpsimd.collective_compute(
                    "AllGather", mybir.AluOpType.bypass,
                    replica_groups=[[0, 1, 2, 3], [4, 5, 6, 7]],
                    ins=[ag_in[cidx].opt()],
                    outs=[ag_out[cidx * D:(cidx + 1) * D, :].opt()],
                )

    blocks = [(qb, h) for qb in range(NQB) for h in range(HPC)]
    last = len(blocks) - 1
    prev = None
    for bi, (qb, h) in enumerate(blocks):
        ht, hr = h // 2, (h % 2) * 64
        q0 = qb * QCH
        if bi == last:
            pv = ps_sc.tile([DK1, QCH], F32, tag="sc", name="pv_last")
        else:
            pv = ps_pv.tile([DK1, QCH], F32, tag="pv")
        cur_sc = []
        for kt in range(NKT):
            ps = ps_sc.tile([128, QCH], F32, tag="sc")
            for u in range(QCH // 512):
                nc.tensor.matmul(
                    ps[:, bass.ts(u, 512)],
                    kh_sb[ht][hr:hr + 64, bass.ts(kt, 128)],
                    qh_sb[ht][hr:hr + 64, q0 + u * 512:q0 + (u + 1) * 512],
                    start=True, stop=True,
                )
            sct = scpool.tile([128, QCH], BF16, tag="sc", name="sc")
            nc.scalar.activation(sct[:], ps[:], EXPF, scale=0.125)
            cur_sc.append(sct)
            if prev is not None:
                ppv, psc, ph, pqb = prev
                for u in range(QCH // 512):
                    nc.tensor.matmul(
                        ppv[:, bass.ts(u, 512)],
                        v_sb[kt][:, ph * DK1:(ph + 1) * DK1],
                        psc[kt][:, bass.ts(u, 512)],
                        start=(kt == 0), stop=(kt == NKT - 1),
                    )
            if bi == last and kt >= 1:
                # self-interleave: last block folds its own PV in with a
                # one-slot lag so the drain after the loop is only kt=15.
                for u in range(QCH // 512):
                    nc.tensor.matmul(
                        pv[:, bass.ts(u, 512)],
                        v_sb[kt - 1][:, h * DK1:(h + 1) * DK1],
                        cur_sc[kt - 1][:, bass.ts(u, 512)],
                        start=(kt - 1 == 0), stop=False,
                    )
        if prev is not None:
            norm_and_out(prev[0], prev[2], prev[3])
        prev = (pv, cur_sc, h, qb)

    # drain: last block only needs kt=15
    ppv, psc, ph, pqb = prev
    for u in range(QCH // 512):
        nc.tensor.matmul(
            ppv[:, bass.ts(u, 512)],
            v_sb[NKT - 1][:, ph * DK1:(ph + 1) * DK1],
            psc[NKT - 1][:, bass.ts(u, 512)],
            start=False, stop=True,
        )
    norm_and_out(ppv, ph, pqb)

    # ---- phase 3: output projection for my token chunk -----------
    # cat rows: pair-select via (pid%4 - pid%2)*512 -> {0, 1024};
    # columns: (pid%2)*512 selects the chunk within the pair.
    pid = nc.sync.partition_id()
    base = nc.sync.snap((pid % 4) * D, donate=False,
                        min_val=0, max_val=3 * D)
    cat_sb = []
    eng = [nc.sync, nc.scalar]
    for kc in range(NKC):
        t = catpool.tile([128, CHUNK], BF16, tag=f"cat{kc}")
        nc.sync.dma_start(
            t[:], ag_out[bass.ds(base + kc * 128, 128), :])
        cat_sb.append(t)
    for qt in range(CHUNK // 128):
        pss = [ps_sc.tile([128, 512], F32, tag="sc", name=f"po{qt}_{i}")
               for i in range(2)]
        for half in range(2):
            nc.tensor.matmul(pss[half][:], ones1[:, 0:128],
                             bo_bf[:, bass.ts(half, 512)],
                             start=True, stop=False)
        for kc in range(NKC):
            for half in range(2):
                nc.tensor.matmul(
                    pss[half][:],
                    cat_sb[kc][:, bass.ts(qt, 128)],
                    wo_sb[kc][:, bass.ts(half, 512)],
                    start=False, stop=(kc == NKC - 1),
                )
        for half in range(2):
            fo = fopool.tile([128, 512], F32, tag="fo")
            nc.scalar.activation(fo[:], pss[half][:],
                                 mybir.ActivationFunctionType.Identity)
            eng[half].dma_start(
                out[bass.ts(qt, 128), bass.ts(half, 512)], fo[:])


def _prep_inputs(q, k, v, Wq, bq, Wk, bk, Wv, bv, Wo, bo):
    """Build the per-core input maps (host-side sharding)."""
    woT = np.ascontiguousarray(Wo.T).astype(NPBF16)
    bo_row = np.ascontiguousarray(bo.reshape(1, D)).astype(NPBF16)
    in_maps = []
    for c in range(N_CORES):
        b, hg = c // 4, c % 4
        fsl = slice(FEAT * hg, FEAT * (hg + 1))
        wv_aug = np.zeros((D, VW), np.float32)
        bv_aug = np.zeros((VW,), np.float32)
        for h in range(HPC):
            rows = slice(FEAT * hg + DK * h, FEAT * hg + DK * (h + 1))
            wv_aug[:, h * DK1:h * DK1 + DK] = Wv[rows, :].T
            bv_aug[h * DK1:h * DK1 + DK] = bv[rows]
            bv_aug[h * DK1 + DK] = 1.0
        in_maps.append({
            "xq": np.ascontiguousarray(q[b].T).astype(NPBF16),
            "xk": np.ascontiguousarray(k[b].T).astype(NPBF16),
            "xv": np.ascontiguousarray(v[b].T).astype(NPBF16),
            "wq": np.ascontiguousarray(Wq[fsl].T).astype(NPBF16),
            "wk": np.ascontiguousarray(Wk[fsl].T).astype(NPBF16),
            "wv": wv_aug.astype(NPBF16),
            "wo": woT,
            "bq": np.ascontiguousarray(
                bq[fsl].reshape(2, 128).T).astype(np.float32),
            "bk": np.ascontiguousarray(
                bk[fsl].reshape(2, 128).T).astype(np.float32),
            "bv": bv_aug.reshape(1, VW).astype(NPBF16),
            "bo": bo_row,  # bf16 row
        })
    return in_maps


def run_sharded(in_maps, trace=False):
    nc = _build_program()
    res = run_bass_kernel_spmd(nc, in_maps, list(range(N_CORES)), trace=trace)
    full = np.empty((B, S, D), np.float32)
    for c in range(N_CORES):
        b, blk = c // 4, c % 4
        full[b, CHUNK * blk:CHUNK * (blk + 1), :] = res.results[c]["out"]
    return full, res


def kernel(q, k, v, Wq, bq, Wk, bk, Wv, bv, Wo, bo):
    args = [np.asarray(x, np.float32) for x in
            (q, k, v, Wq, bq, Wk, bk, Wv, bv, Wo, bo)]
    in_maps = _prep_inputs(*args)
    full, _ = run_sharded(in_maps)
    return full

